# revision 14
# baseline (speedup 1.0000x reference)
"""ClusterGCN + 2x GAT message-passing kernel for 8 Trainium2 NeuronCores.

Strategy (dst-sharded, one SPMD program):
  - Nodes are permuted into 784 tiles of 128 slots, load-balanced so every
    tile has (nearly) the same number of incoming edges (self-loops added).
    Cores own 98 consecutive tiles each.
  - Per layer, each core gathers the rows of its incoming messages from a
    replicated node table in its HBM via per-column indirect DMA (the Q7
    descriptor-generation rate is ~8ns/row regardless of batching, so the
    gather is organized for zero padding), reduces them per dst tile with
    0/1 selection-matrix matmuls accumulated in PSUM, and applies the layer
    transform in feature-major (transposed) space.
  - Per-edge s_dst: the per-tile s_dst column is row-replicated with one PE
    transpose-of-broadcast (tile_scatter_add trick), then expanded to edge
    positions by reusing the selection matrix: sde2 = reduce_c(sel * sdfull)
    on the Vector engine. No extra DMA descriptors, no per-column selT
    transposes/matmuls.
  - exp(leaky_relu(l)) = max(exp(l), exp(0.2*l)): both exps on the Scalar
    engine (scale fused). s_src rides in the gathered row as bf16 hi/lo.
  - PSUM evacuation / scaling copies run on the Scalar engine.
  - Between layers the per-core z-tables (h @ W with packed attention
    scalars) are AllGathered so every core can gather arbitrary source rows.
"""

import os
import sys

sys.path.insert(0, "/opt/trn_rl_repo")
os.environ.setdefault("NEURON_RT_RESET_CORES", "1")

import numpy as np

import concourse.bacc as bacc
import concourse.bass as bass
import concourse.mybir as mybir
import concourse.tile as tile
from concourse.bass_utils import run_bass_kernel_spmd

# ---- problem constants (hardcoded per contest rules) ----
N = 100000
E = 1600000
FIN = 64
HID = 64
FOUT = 32
NEG = 0.2

P = 128
NCORES = 8
TILES_PER_CORE = 98
T_ALL = NCORES * TILES_PER_CORE  # 784
NPC = TILES_PER_CORE * P  # 12544 nodes per core
NP_ALL = T_ALL * P  # 100352 padded node count

FW1 = 68  # z1 row: z(64) | 1.0 | s_hi | s_lo | pad
FW2 = 36  # z2 row: z(32) | 1.0 | s_hi | s_lo | pad

F32 = mybir.dt.float32
BF16 = mybir.dt.bfloat16
I32 = mybir.dt.int32
AF = mybir.ActivationFunctionType
OP = mybir.AluOpType

_cache = {}
last_result = None


def _bf16(a):
    import ml_dtypes

    return np.asarray(a, dtype=ml_dtypes.bfloat16)


# ----------------------------------------------------------------------------
# host-side preprocessing
# ----------------------------------------------------------------------------
def _balance_tiles(deg):
    """Assign each of NP_ALL nodes to one of T_ALL tiles (128 slots each) so
    that per-tile total in-degree is near-uniform. Returns perm arrays."""
    import heapq

    order = np.argsort(-deg, kind="stable")
    heap = [(0, t) for t in range(T_ALL)]
    heapq.heapify(heap)
    counts = np.zeros(T_ALL, np.int64)
    loads = np.zeros(T_ALL, np.int64)
    tile_of = np.empty(NP_ALL, np.int32)
    slot_of = np.empty(NP_ALL, np.int32)
    for n in order:
        while True:
            load, t = heapq.heappop(heap)
            if counts[t] < P:
                break
        tile_of[n] = t
        slot_of[n] = counts[t]
        counts[t] += 1
        loads[t] += deg[n]
        if counts[t] < P:
            heapq.heappush(heap, (loads[t], t))
    return tile_of, slot_of, int(loads.max())


def _preprocess(x, edge_index):
    src = np.asarray(edge_index[0], np.int64)
    dst = np.asarray(edge_index[1], np.int64)
    loops = np.arange(NP_ALL, dtype=np.int64)
    src_all = np.concatenate([src, loops])
    dst_all = np.concatenate([dst, loops])
    deg = np.bincount(dst_all, minlength=NP_ALL)  # includes self-loop

    tile_of, slot_of, max_load = _balance_tiles(deg)
    ku = (max_load + P - 1) // P
    gid = tile_of.astype(np.int64) * P + slot_of  # node -> permuted row

    m_src = gid[src_all]
    m_tile = tile_of[dst_all].astype(np.int64)
    m_slot = slot_of[dst_all].astype(np.int64)

    # column 0 of every tile holds the 128 self-loop messages slot-aligned;
    # edge messages fill columns 1..ku-1
    is_loop_m = np.concatenate(
        [np.zeros(len(src), bool), np.ones(NP_ALL, bool)]
    )
    order = np.lexsort((~is_loop_m, m_tile))
    m_src, m_tile, m_slot, is_loop_m = (
        m_src[order],
        m_tile[order],
        m_slot[order],
        is_loop_m[order],
    )
    tile_counts = np.bincount(m_tile, minlength=T_ALL)
    tile_starts = np.concatenate([[0], np.cumsum(tile_counts)[:-1]])
    pos = np.arange(len(m_src)) - tile_starts[m_tile]
    mp = np.where(is_loop_m, m_slot, pos % P)
    mc = np.where(is_loop_m, 0, pos // P)

    cols = TILES_PER_CORE * ku
    midx = np.zeros((NCORES, P, cols), np.int32)
    mloc = np.full((NCORES, P, cols), -1.0, np.float32)
    core = m_tile // TILES_PER_CORE
    tl = m_tile % TILES_PER_CORE
    col = tl * ku + mc
    midx[core, mp, col] = m_src
    mloc[core, mp, col] = m_slot

    deg_inv = (1.0 / np.maximum(deg, 1.0)).astype(np.float32)
    deginv_core = deg_inv[np.argsort(gid)].reshape(NCORES, TILES_PER_CORE, P)
    deginv_core = np.ascontiguousarray(np.transpose(deginv_core, (0, 2, 1)))

    xp = np.zeros((NP_ALL, FIN), np.float32)
    xp[gid[:N]] = np.asarray(x, np.float32)
    return dict(
        ku=int(ku),
        midx=midx,
        mloc=mloc,
        deginv=deginv_core,
        xp=xp,
        gid=gid,
    )


# ----------------------------------------------------------------------------
# device program
# ----------------------------------------------------------------------------
def _padP(a):
    out = np.zeros((P, a.shape[1]), a.dtype)
    out[: a.shape[0]] = a
    return out


def _hilo(v):
    hi = _bf16(np.asarray(v, np.float32))
    lo = _bf16(np.asarray(v, np.float32) - np.asarray(hi, np.float32))
    return hi, lo


def _build_program(ku):
    phases = int(os.environ.get("KERNEL_PHASES", "3"))
    nc = bacc.Bacc()
    cols = TILES_PER_CORE * ku

    CF = 492
    CB = ku * P + cols
    CI = cols
    xe_in = nc.declare_dram_parameter(
        "xe", [TILES_PER_CORE, P, ku * FIN], BF16, isOutput=False
    )
    xlocT = nc.declare_dram_parameter("xlocT", [FIN, NPC], F32, isOutput=False)
    cf_in = nc.declare_dram_parameter("constf", [P, CF], F32, isOutput=False)
    cb_in = nc.declare_dram_parameter("constb", [P, CB], BF16, isOutput=False)
    ci_in = nc.declare_dram_parameter("consti", [P, CI], I32, isOutput=False)
    outloc = nc.declare_dram_parameter("outloc", [NPC, FOUT], F32, isOutput=True)

    z1loc = nc.dram_tensor("z1loc", [NPC, FW1], BF16)
    z1tab = nc.dram_tensor("z1tab", [NP_ALL, FW1], BF16, addr_space="Shared")
    z2loc = nc.dram_tensor("z2loc", [NPC, FW2], BF16)
    z2tab = nc.dram_tensor("z2tab", [NP_ALL, FW2], BF16, addr_space="Shared")
    sd1 = nc.dram_tensor("sd1", [NPC, 1], F32)
    sd2 = nc.dram_tensor("sd2", [NPC, 1], F32)

    groups = [list(range(NCORES))]

    with tile.TileContext(nc) as tc:
        with (
            tc.tile_pool(name="const", bufs=1) as cpool,
            tc.tile_pool(name="sbuf", bufs=4) as pool,
            tc.tile_pool(name="gath", bufs=14) as gpool,
            tc.tile_pool(name="psum", bufs=2, space="PSUM") as pacc,
            tc.tile_pool(name="psum1", bufs=1, space="PSUM") as ptp,
        ):
            def cload(ap, shape, dt, tag):
                t = cpool.tile(shape, dt, tag=tag)
                nc.sync.dma_start(out=t[:], in_=ap)
                return t

            cf = cload(cf_in[:, :], [P, CF], F32, tag="cf")
            cb = cload(cb_in[:, :], [P, CB], BF16, tag="cb")
            ci = cload(ci_in[:, :], [P, CI], I32, tag="ci")
            ident_t = cf[:, 0:128]
            dinv_t = cf[:, 128:226]
            b2r_t = cf[:, 226:258]
            bout_t = cf[:HID, 258:259]
            b1c_t = cf[:HID, 259:260]
            a1_t = cf[:HID, 260:264]
            a2_t = cf[:FOUT, 264:268]
            wout_t = cf[:FIN, 268:332]
            wroot_t = cf[:FIN, 332:396]
            w1_t = cf[:HID, 396:460]
            w2_t = cf[:HID, 460:492]
            iotak_t = cb[:, 0 : ku * P]
            mloc_t = cb[:, ku * P :]
            midx_t = ci[:, :]

            sdcol = cpool.tile([P, TILES_PER_CORE], F32, tag="sdcol")

            def sel_build(ti):
                """0/1 bf16 selection [P, ku, P] for tile ti."""
                sel = pool.tile([P, ku, P], BF16, tag="sel")
                nc.vector.tensor_tensor(
                    out=sel[:, :, :],
                    in0=mloc_t[:, ti * ku : (ti + 1) * ku, None].to_broadcast(
                        [P, ku, P]
                    ),
                    in1=iotak_t[:].rearrange("p (k c) -> p k c", k=ku),
                    op=OP.is_equal,
                )
                return sel

            def pack_from_T(hT_sb, w_t, a_t, fo, fw, zloc, ti):
                """Feature-major f32 activations hT_sb [fi, P] for tile ti ->
                z = h @ W, s_src / s_dst = z @ a, packed z-row to zloc,
                s_dst column stashed in sdcol."""
                zT_ps = ptp.tile([fo, P], F32, tag="zT")
                nc.tensor.matmul(
                    out=zT_ps[:], lhsT=w_t, rhs=hT_sb, start=True, stop=True
                )
                zT_sb = pool.tile([fo, P], F32, tag="zTsb")
                nc.scalar.copy(out=zT_sb[:], in_=zT_ps[:])
                sc_ps = ptp.tile([P, 4], F32, tag="sc")
                nc.tensor.matmul(
                    out=sc_ps[:], lhsT=zT_sb[:, :], rhs=a_t, start=True, stop=True
                )
                sc_sb = pool.tile([P, 4], F32, tag="sc_sb")
                nc.scalar.copy(out=sc_sb[:], in_=sc_ps[:, :])
                ssrc = pool.tile([P, 1], F32, tag="ssrc")
                nc.vector.tensor_tensor(
                    out=ssrc[:], in0=sc_sb[:, 0:1], in1=sc_sb[:, 1:2], op=OP.add
                )
                nc.vector.tensor_tensor(
                    out=sdcol[:, ti : ti + 1],
                    in0=sc_sb[:, 2:3],
                    in1=sc_sb[:, 3:4],
                    op=OP.add,
                )
                zr_ps = ptp.tile([P, fo], F32, tag="zr")
                nc.tensor.transpose(
                    out=zr_ps[:], in_=zT_sb[:, :], identity=ident_t[:fo, 0:fo]
                )
                zrow = pool.tile([P, fw], BF16, tag="zrow")
                nc.scalar.copy(out=zrow[:, 0:fo], in_=zr_ps[:, :])
                nc.vector.memset(zrow[:, fo : fo + 1], 1.0)
                nc.vector.memset(zrow[:, fo + 3 : fw], 0.0)
                nc.scalar.copy(out=zrow[:, fo + 1 : fo + 2], in_=ssrc[:, :])
                shi_f = pool.tile([P, 1], F32, tag="shif")
                nc.scalar.copy(out=shi_f[:], in_=zrow[:, fo + 1 : fo + 2])
                nc.vector.tensor_tensor(
                    out=zrow[:, fo + 2 : fo + 3],
                    in0=ssrc[:, :],
                    in1=shi_f[:, :],
                    op=OP.subtract,
                )
                nc.sync.dma_start(
                    out=zloc[ti * P : (ti + 1) * P, :], in_=zrow[:, :]
                )

            # ================= Layer 1: ClusterGCN =================
            for ti in range(TILES_PER_CORE):
                    msg = gpool.tile([P, ku * FIN], BF16, tag="msg1")
                    nc.sync.dma_start(out=msg[:, :], in_=xe_in[ti, :, :])
                    sel = sel_build(ti)
                    acc = pacc.tile([P, FIN], F32, tag="acc")
                    for k in range(ku):
                        nc.tensor.matmul(
                            out=acc[:],
                            lhsT=sel[:, k, :],
                            rhs=msg[:, k * FIN : (k + 1) * FIN],
                            start=(k == 0),
                            stop=(k == ku - 1),
                        )
                    agg = pool.tile([P, FIN], F32, tag="agg")
                    nc.scalar.mul(agg[:], acc[:, :], dinv_t[:, ti : ti + 1])
                    xT_sb = pool.tile([FIN, P], F32, tag="xT")
                    nc.sync.dma_start(
                        out=xT_sb[:], in_=xlocT[:, ti * P : (ti + 1) * P]
                    )
                    aT_ps = ptp.tile([FIN, P], F32, tag="tp")
                    nc.tensor.transpose(out=aT_ps[:], in_=agg[:, :], identity=ident_t)
                    aT_sb = pool.tile([FIN, P], F32, tag="aT")
                    nc.scalar.copy(out=aT_sb[:], in_=aT_ps[:])
                    hT_ps = ptp.tile([HID, P], F32, tag="hTp")
                    nc.tensor.matmul(
                        out=hT_ps[:], lhsT=wout_t, rhs=aT_sb[:, :],
                        start=True, stop=False,
                    )
                    nc.tensor.matmul(
                        out=hT_ps[:], lhsT=wroot_t, rhs=xT_sb[:, :],
                        start=False, stop=True,
                    )
                    h1T_sb = pool.tile([HID, P], F32, tag="h1T")
                    nc.scalar.activation(
                        out=h1T_sb[:], in_=hT_ps[:], func=AF.Relu, bias=bout_t
                    )
                    pack_from_T(h1T_sb[:, :], w1_t, a1_t, HID, FW1, z1loc, ti)
            nc.sync.dma_start(
                out=sd1[:, :].rearrange("(p t) one -> p (t one)", p=P),
                in_=sdcol[:, :],
            )
            if phases >= 1:
                tc.strict_bb_all_engine_barrier()
                nc.gpsimd.collective_compute(
                    "AllGather",
                    OP.bypass,
                    replica_groups=groups,
                    ins=[z1loc[:, :]],
                    outs=[z1tab[:, :]],
                )
                tc.strict_bb_all_engine_barrier()

            # ================= Layers 2 & 3: GAT =================
            def gat_layer(ztab, zself, sdt, fw, fo, w_t, a_t, zloc_next, fw_next, sd_next, last):
                fz = fw - 4
                sdl = cpool.tile([P, TILES_PER_CORE], F32, tag=f"sdl{fw}")
                nc.sync.dma_start(
                    out=sdl[:],
                    in_=sdt[:, :].rearrange("(p t) one -> p (t one)", p=P),
                )
                for ti in range(TILES_PER_CORE):
                        msg = gpool.tile([P, ku, fw], BF16, tag="msg2")
                        for k in range(1, ku):
                            nc.gpsimd.indirect_dma_start(
                                out=msg[:, k, :],
                                out_offset=None,
                                in_=ztab[:, :],
                                in_offset=bass.IndirectOffsetOnAxis(
                                    ap=midx_t[:, ti * ku + k : ti * ku + k + 1],
                                    axis=0,
                                ),
                            )
                        # column 0 = slot-aligned self-loop rows: contiguous
                        # (issued after the indirect gathers so the Q7 stream
                        # never waits on the HWDGE write to this tile)
                        nc.sync.dma_start(
                            out=msg[:, 0, :],
                            in_=zself[ti * P : (ti + 1) * P, :],
                        )
                        sel = sel_build(ti)
                        # s_dst row-replication: transpose of broadcast column
                        sdf_ps = ptp.tile([P, P], F32, tag="sdf")
                        nc.tensor.transpose(
                            out=sdf_ps[:],
                            in_=sdl[:, ti : ti + 1].to_broadcast([P, P]),
                            identity=ident_t,
                        )
                        sdf_sb = pool.tile([P, P], BF16, tag="sdfb")
                        nc.scalar.copy(out=sdf_sb[:], in_=sdf_ps[:])
                        # sde2[p,k] = sum_c sel[p,k,c] * s_dst[c]
                        sds = pool.tile([P, ku, P], BF16, tag="sds")
                        nc.vector.tensor_tensor(
                            out=sds[:, :, :],
                            in0=sel[:, :, :],
                            in1=sdf_sb[:, None, :].to_broadcast([P, ku, P]),
                            op=OP.mult,
                        )
                        sde2 = pool.tile([P, ku], F32, tag="sde2")
                        nc.vector.tensor_reduce(
                            out=sde2[:, :],
                            in_=sds[:, :, :],
                            axis=mybir.AxisListType.X,
                            op=OP.add,
                        )
                        # logits l = s_src(hi+lo) + s_dst
                        l = pool.tile([P, ku], F32, tag="l")
                        nc.vector.tensor_tensor(
                            out=l[:],
                            in0=msg[:, :, fz + 1],
                            in1=msg[:, :, fz + 2],
                            op=OP.add,
                        )
                        nc.vector.tensor_tensor(
                            out=l[:], in0=l[:], in1=sde2[:, :], op=OP.add
                        )
                        # w = exp(leaky_relu(l)) = max(exp(l), exp(0.2*l))
                        w1e = pool.tile([P, ku], F32, tag="w1")
                        nc.scalar.activation(out=w1e[:], in_=l[:], func=AF.Exp)
                        w2e = pool.tile([P, ku], F32, tag="w2")
                        nc.scalar.activation(
                            out=w2e[:], in_=l[:], func=AF.Exp, scale=float(NEG)
                        )
                        wb = pool.tile([P, ku], BF16, tag="wb")
                        nc.vector.tensor_tensor(
                            out=wb[:], in0=w1e[:], in1=w2e[:], op=OP.max
                        )
                        # weighted messages (+denominator column fz)
                        mp = pool.tile([P, ku, fz + 1], BF16, tag="mp")
                        nc.vector.tensor_tensor(
                            out=mp[:, :, :],
                            in0=msg[:, :, 0 : fz + 1],
                            in1=wb[:, :, None].to_broadcast([P, ku, fz + 1]),
                            op=OP.mult,
                        )
                        acc = pacc.tile([P, fz + 1], F32, tag="acc")
                        for k in range(ku):
                            nc.tensor.matmul(
                                out=acc[:],
                                lhsT=sel[:, k, :],
                                rhs=mp[:, k, :],
                                start=(k == 0),
                                stop=(k == ku - 1),
                            )
                        den = pool.tile([P, 1], F32, tag="den")
                        nc.vector.tensor_scalar(
                            out=den[:], in0=acc[:, fz : fz + 1], scalar1=1e-30,
                            scalar2=None, op0=OP.max,
                        )
                        rec = pool.tile([P, 1], F32, tag="rec")
                        nc.vector.reciprocal(out=rec[:], in_=den[:])
                        if last:
                            h = pool.tile([P, fz], F32, tag="h")
                            nc.scalar.mul(h[:], acc[:, 0:fz], rec[:, :])
                            nc.vector.tensor_tensor(
                                out=h[:], in0=h[:], in1=b2r_t, op=OP.add
                            )
                            nc.sync.dma_start(
                                out=outloc[ti * P : (ti + 1) * P, :], in_=h[:, :]
                            )
                        else:
                            hsc = pool.tile([P, fz], F32, tag="hsc")
                            nc.scalar.mul(hsc[:], acc[:, 0:fz], rec[:, :])
                            hT_ps = ptp.tile([fz, P], F32, tag="tp")
                            nc.tensor.transpose(
                                out=hT_ps[:], in_=hsc[:, :], identity=ident_t
                            )
                            hT_sb = pool.tile([fz, P], F32, tag="h1T")
                            nc.scalar.activation(
                                out=hT_sb[:], in_=hT_ps[:], func=AF.Relu,
                                bias=b1c_t,
                            )
                            pack_from_T(
                                hT_sb[:, :], w_t, a_t, fw_next - 4, fw_next,
                                zloc_next, ti,
                            )
                if not last:
                    nc.sync.dma_start(
                        out=sd_next[:, :].rearrange("(p t) one -> p (t one)", p=P),
                        in_=sdcol[:, :],
                    )

            if phases >= 2:
                gat_layer(z1tab, z1loc, sd1, FW1, HID, w2_t, a2_t, z2loc, FW2, sd2, False)
            if phases >= 3:
                tc.strict_bb_all_engine_barrier()
                nc.gpsimd.collective_compute(
                    "AllGather",
                    OP.bypass,
                    replica_groups=groups,
                    ins=[z2loc[:, :]],
                    outs=[z2tab[:, :]],
                )
                tc.strict_bb_all_engine_barrier()
                gat_layer(z2tab, z2loc, sd2, FW2, FOUT, None, None, None, None, None, True)
            if phases < 3:
                for ti in range(TILES_PER_CORE):
                    zt = pool.tile([P, FOUT], F32, tag="h")
                    nc.vector.memset(zt[:], 0.0)
                    nc.sync.dma_start(out=outloc[ti * P : (ti + 1) * P, :], in_=zt[:, :])

    nc.finalize()
    return nc


# ----------------------------------------------------------------------------
# entry point
# ----------------------------------------------------------------------------
def kernel(
    x,
    edge_index,
    W_out,
    b_out,
    W_root,
    W1,
    a_src1,
    a_dst1,
    b1,
    W2,
    a_src2,
    a_dst2,
    b2,
    training=0,
    **_unused,
):
    pre = _preprocess(x, edge_index)
    ku = pre["ku"]
    _key = (ku, os.environ.get("KERNEL_PHASES", "3"))
    if _key not in _cache:
        _cache[_key] = _build_program(ku)
    nc = _cache[_key]

    iota = np.tile(np.arange(P, dtype=np.float32), (P, 1))
    iotak = np.tile(iota, (1, ku))
    ident = np.eye(P, dtype=np.float32)

    a1hi, a1lo = _hilo(np.asarray(a_src1, np.float32))
    a1dhi, a1dlo = _hilo(np.asarray(a_dst1, np.float32))
    a2hi, a2lo = _hilo(np.asarray(a_src2, np.float32))
    a2dhi, a2dlo = _hilo(np.asarray(a_dst2, np.float32))
    a1 = np.stack(
        [np.float32(a1hi), np.float32(a1lo), np.float32(a1dhi), np.float32(a1dlo)], 1
    )
    a2 = np.stack(
        [np.float32(a2hi), np.float32(a2lo), np.float32(a2dhi), np.float32(a2dlo)], 1
    )

    xp = pre["xp"]
    xp_b = _bf16(xp)
    in_maps = []
    for c in range(NCORES):
        mi = pre["midx"][c].reshape(P, TILES_PER_CORE, ku).astype(np.int64)
        xe = np.ascontiguousarray(
            np.transpose(xp_b[mi], (1, 0, 2, 3))
        ).reshape(TILES_PER_CORE, P, ku * FIN)
        in_maps.append(
            {
                "xe": xe,
                "xlocT": np.ascontiguousarray(xp[c * NPC : (c + 1) * NPC].T),
                "constf": np.concatenate(
                    [
                        ident,
                        pre["deginv"][c],
                        np.tile(np.asarray(b2, np.float32), (P, 1)),
                        _padP(np.asarray(b_out, np.float32).reshape(HID, 1)),
                        _padP(np.asarray(b1, np.float32).reshape(HID, 1)),
                        _padP(a1),
                        _padP(a2),
                        _padP(np.asarray(W_out, np.float32)),
                        _padP(np.asarray(W_root, np.float32)),
                        _padP(np.asarray(W1, np.float32)),
                        _padP(np.asarray(W2, np.float32)),
                    ],
                    axis=1,
                ),
                "constb": np.concatenate(
                    [_bf16(iotak), _bf16(pre["mloc"][c])], axis=1
                ),
                "consti": pre["midx"][c],
            }
        )

    trace = bool(os.environ.get("BASS_TRACE"))
    res = run_bass_kernel_spmd(
        nc, in_maps, list(range(NCORES)), trace=trace
    )
    global last_result
    last_result = res
    out_p = np.concatenate([res.results[c]["outloc"] for c in range(NCORES)], 0)
    out = out_p[pre["gid"][:N]]
    return np.asarray(out, np.float32)
